# revision 41
# baseline (speedup 1.0000x reference)
"""GCN forward (4x GCNConv + linear head) on 8 Trainium2 NeuronCores.

Sharding: nodes are partitioned across the 8 cores (2048 rows each).
Per layer: AllGather of node-major activations (bf16), per-dst-tile
dma_gather of neighbor rows, scatter-matmul S.T @ gathered (PSUM f32
accumulate), PE-transpose to feature-major, dense GEMM.

Uses the algebraic identity A_hat (h W) = (A_hat h) W to aggregate in the
cheaper feature dimension per layer (layers 1-3 aggregate first, layer 4
aggregates after its GEMM).

Layer GEMM forms:
  - L1/L2 ("form A"): lhsT = transposed-aggregation block [feat,node],
    rhs = full weight panel [feat, fo] resident in SBUF.  Output lands
    node-major directly (no output transposes), bias is pre-loaded into
    PSUM via a K=1 ones-row matmul, relu fused in the ScalarE copy-out.
    Weights are streamed exactly once per layer.
  - L3 ("form B"): lhsT = weight columns, rhs = aggregation (both
    1024-node halves resident), output feature-major h4T for layer 4.
  - L4 ("form B"): lhsT = W4 columns, rhs = h4T node-slices (pair
    resident), node-major output via PE transposes, AllGather, then the
    layer-4 aggregation writes bias+relu'd strips straight into SBUF
    tiles that the output head consumes as its rhs (no DRAM roundtrip).
"""

import os

import numpy as np
import ml_dtypes

try:  # persistent compilation cache: skip walrus recompile across processes
    import jax
    jax.config.update("jax_compilation_cache_dir",
                      os.path.expanduser("~/.cache/jax_gcn_kernel"))
    jax.config.update("jax_persistent_cache_min_compile_time_secs", 10)
except Exception:
    pass

import concourse.bass as bass
import concourse.mybir as mybir
import concourse.tile as tile
from concourse import bacc
from concourse.bass_utils import run_bass_kernel_spmd
from concourse.masks import make_identity

N = 16384
E = 65536
NCORES = 8
NL = N // NCORES            # 2048 nodes per core
NT = NL // 128              # 16 dst tiles per core
DIMS = [512, 1024, 2048, 4096, 2048]
C = 1000
CPAD = 1024                 # output classes padded to a multiple of 128
AG_SPLIT = 4                # AllGather emitted per quarter-shard

BF = mybir.dt.bfloat16
F32 = mybir.dt.float32
I16 = mybir.dt.int16
RELU = mybir.ActivationFunctionType.Relu
COPY = mybir.ActivationFunctionType.Copy
NPBF = ml_dtypes.bfloat16

_CACHE = {}


# ----------------------------------------------------------------------------
# Device program
# ----------------------------------------------------------------------------

def _build(chunks):
    nc = bacc.Bacc("TRN2", target_bir_lowering=False, num_devices=NCORES)

    # ---- kernel I/O ----
    x_bf = nc.dram_tensor("x_bf", [N, DIMS[0]], BF, kind="ExternalInput")
    idx_d = nc.dram_tensor("idx", [128, NT, chunks * 8], I16, kind="ExternalInput")
    s_d = nc.dram_tensor("smat", [128, NT, chunks, 128], BF, kind="ExternalInput")
    # form-A weight panels [feat-part, fa/128, fo] for layers 1-2
    wa_d = [
        nc.dram_tensor("wa0", [128, DIMS[0] // 128, DIMS[1]], BF,
                       kind="ExternalInput"),
        nc.dram_tensor("wa1", [128, DIMS[1] // 128, DIMS[2]], BF,
                       kind="ExternalInput"),
    ]
    # bias rows [1, fo] for layers 1-2 (preloaded into PSUM via K=1 matmul)
    brow_d = [
        nc.dram_tensor("brow0", [1, DIMS[1]], BF, kind="ExternalInput"),
        nc.dram_tensor("brow1", [1, DIMS[2]], BF, kind="ExternalInput"),
    ]
    # form-B weight columns [part, fo/128, fa/128, 128] for L3, L4, head
    wb_dims = [(DIMS[2], DIMS[3]), (DIMS[3], DIMS[4]), (DIMS[4], CPAD)]
    wb_d = []
    bcol_d = []
    for i, (fa, fo) in enumerate(wb_dims):
        wb_d.append(nc.dram_tensor(f"wb{i}", [128, fo // 128, fa // 128, 128],
                                   BF, kind="ExternalInput"))
        bcol_d.append(nc.dram_tensor(f"bcol{i}", [128, fo // 128], F32,
                                     kind="ExternalInput"))
    outT = nc.dram_tensor("outT", [CPAD, NL], F32, kind="ExternalOutput")

    # ---- internal DRAM ----
    h2_nm = nc.dram_tensor("h2_nm", [NL, DIMS[1]], BF)
    h2_full = nc.dram_tensor("h2_full", [N, DIMS[1]], BF, addr_space="Shared")
    h3_nm = nc.dram_tensor("h3_nm", [NL, DIMS[2]], BF)
    h3_full = nc.dram_tensor("h3_full", [N, DIMS[2]], BF, addr_space="Shared")
    h4T = nc.dram_tensor("h4T", [DIMS[3], NL], BF)
    m4_nm = nc.dram_tensor("m4_nm", [NL, DIMS[4]], BF)
    m4_full = nc.dram_tensor("m4_full", [N, DIMS[4]], BF, addr_space="Shared")

    rg = [list(range(NCORES))]

    with tile.TileContext(nc) as tc:
        with (
            tc.tile_pool(name="const", bufs=1) as p_const,
            tc.tile_pool(name="gath", bufs=2) as p_gath,
            tc.tile_pool(name="rhs", bufs=2) as p_rhs,
            tc.tile_pool(name="wrhs", bufs=1) as p_wrhs,
            tc.tile_pool(name="aggw", bufs=4 if chunks <= 5 else 3) as p_aggw,
            tc.tile_pool(name="wcol", bufs=2 if chunks <= 5 else 1) as p_w,
            tc.tile_pool(name="nm", bufs=4 if chunks <= 5 else 2) as p_nm,
            tc.tile_pool(name="nmst", bufs=2 if chunks <= 5 else 1) as p_nmst,
            tc.tile_pool(name="ht", bufs=8 if chunks <= 5 else 2) as p_ht,
            tc.tile_pool(name="of", bufs=1) as p_of,
            tc.tile_pool(name="tp", bufs=2, space="PSUM") as p_tp,
            tc.tile_pool(name="aggps", bufs=2, space="PSUM") as p_agg,
            tc.tile_pool(name="gemmps", bufs=2, space="PSUM") as p_gemm,
        ):
            # ---- constants ----
            idx_sb = p_const.tile([128, NT, chunks * 8], I16)
            nc.sync.dma_start(out=idx_sb[:], in_=idx_d[:])
            s_sb = p_const.tile([128, NT, chunks, 128], BF)
            nc.sync.dma_start(out=s_sb[:], in_=s_d[:])
            ident = p_const.tile([128, 128], BF)
            make_identity(nc, ident[:])
            ones1 = p_const.tile([1, 128], BF)
            nc.vector.memset(ones1[:], 1.0)
            brow_sb = []
            for l in range(2):
                bt = p_const.tile([1, DIMS[l + 1]], BF, tag=f"brow{l}")
                nc.sync.dma_start(out=bt[:], in_=brow_d[l][:])
                brow_sb.append(bt)
            bcol_sb = []
            for i, (fa, fo) in enumerate(wb_dims):
                bt = p_const.tile([128, fo // 128], F32, tag=f"bcol{i}")
                nc.sync.dma_start(out=bt[:], in_=bcol_d[i][:])
                bcol_sb.append(bt)

            def allgather_rows(nm, full, k, nsplit):
                # AG of shard rows [k*NL/s,(k+1)*NL/s) -> full rows
                # [k*N/s,(k+1)*N/s). Gather indices are host-remapped to this
                # block-of-row-blocks layout.
                rl = NL // nsplit
                gl = N // nsplit
                nc.gpsimd.collective_compute(
                    "AllGather", mybir.AluOpType.bypass, replica_groups=rg,
                    ins=[nm.ap()[k * rl:(k + 1) * rl, :].opt()],
                    outs=[full.ap()[k * gl:(k + 1) * gl, :].opt()],
                )

            def agg_group(src_ap, fa, t0, nt, sinks):
                """Aggregate dst tiles t0..t0+nt-1 with a single dma_gather.

                sinks[i](fb, tp) receives the transposed 128x128 PSUM block
                for features [fb*128,(fb+1)*128) of tile t0+i.
                """
                gath = p_gath.tile([128, nt * chunks, fa], BF, tag="gath")
                nc.gpsimd.dma_gather(
                    gath[:], src_ap, idx_sb[:, t0:t0 + nt, :],
                    nt * chunks * 128, nt * chunks * 128, fa,
                    single_packet=False,
                )
                for i in range(nt):
                    t = t0 + i
                    for q in range(fa // 512):
                        aps = p_agg.tile([128, 512], F32, tag="aggps")
                        for c in range(chunks):
                            nc.tensor.matmul(
                                out=aps[:],
                                lhsT=s_sb[:, t, c, :],
                                rhs=gath[:, i * chunks + c,
                                         q * 512:(q + 1) * 512],
                                start=(c == 0), stop=(c == chunks - 1),
                            )
                        nm = p_nm.tile([128, 512], BF, tag="aggnm")
                        nc.vector.tensor_copy(out=nm[:], in_=aps[:])
                        for bb in range(4):
                            fb = q * 4 + bb
                            tp = p_tp.tile([128, 128], BF, tag="tp")
                            nc.tensor.transpose(
                                out=tp[:], in_=nm[:, bb * 128:(bb + 1) * 128],
                                identity=ident[:],
                            )
                            sinks[i](fb, tp)

            def layer_A(src_ap, fa, fo, li, out_nm, out_full):
                """Form-A layer (L1/L2): rolling agg -> GEMM per node tile.

                Gathers run two dst tiles per dma_gather.  The half-shard
                AllGather triggers are emitted only after the last gather of
                the layer so a blocked trigger never stalls this layer's
                gather queue.
                """
                w_rhs = p_wrhs.tile([128, fa // 128, fo], BF, tag="wrhs",
                                    name=f"wrhs{li}")
                nc.sync.dma_start(out=w_rhs[:], in_=wa_d[li][:])
                nkb = fa // 128
                nfh = fo // 1024

                def gemm_tile(t):
                    for fh in range(nfh):
                        gps = p_gemm.tile([128, 1024], F32, tag="g")
                        for j in range(2):
                            nc.tensor.matmul(
                                out=gps[:, j * 512:(j + 1) * 512],
                                lhsT=ones1[:],
                                rhs=brow_sb[li][:, fh * 1024 + j * 512:
                                                fh * 1024 + (j + 1) * 512],
                                start=True, stop=False,
                            )
                        for kb in range(nkb):
                            for j in range(2):
                                nc.tensor.matmul(
                                    out=gps[:, j * 512:(j + 1) * 512],
                                    lhsT=aggT[t][:, kb, :],
                                    rhs=w_rhs[:, kb, fh * 1024 + j * 512:
                                              fh * 1024 + (j + 1) * 512],
                                    start=False, stop=(kb == nkb - 1),
                                )
                        for j in range(2):
                            ht = p_ht.tile([128, 512], BF, tag="ht")
                            nc.scalar.activation(
                                out=ht[:], in_=gps[:, j * 512:(j + 1) * 512],
                                func=RELU)
                            nc.sync.dma_start(
                                out=out_nm.ap()[t * 128:(t + 1) * 128,
                                                fh * 1024 + j * 512:
                                                fh * 1024 + (j + 1) * 512],
                                in_=ht[:],
                            )

                aggT = {}

                def make_sink(t):
                    at = p_aggw.tile([128, nkb, 128], BF, tag="aggw")
                    aggT[t] = at
                    return lambda fb, tp: nc.vector.tensor_copy(
                        out=at[:, fb, :], in_=tp[:])

                # Quarter-k AG triggers are emitted on the gpsimd queue only
                # once the producing GEMM tile (4k+3) is safely complete, so
                # a waiting trigger never stalls the remaining gathers.
                for t in range(NT):
                    agg_group(src_ap, fa, t, 1, [make_sink(t)])
                    if t >= 2:
                        gemm_tile(t - 2)
                    if t == 11:
                        allgather_rows(out_nm, out_full, 0, 4)
                    elif t == 13:
                        allgather_rows(out_nm, out_full, 1, 4)
                    elif t == 15:
                        allgather_rows(out_nm, out_full, 2, 4)
                gemm_tile(NT - 2)
                gemm_tile(NT - 1)
                allgather_rows(out_nm, out_full, 3, 4)

            def layer_L3(src_ap):
                fa, fo = DIMS[2], DIMS[3]
                nkb = fa // 128

                def agg_half(half):
                    at = p_rhs.tile([128, nkb, 1024], BF, tag="rhs",
                                    name=f"aggT3_{half}")
                    for tt in range(8):
                        t = half * 8 + tt
                        agg_group(src_ap, fa, t, 1,
                                  [lambda fb, tp, at=at, tt=tt:
                                   nc.vector.tensor_copy(
                                       out=at[:, fb, tt * 128:(tt + 1) * 128],
                                       in_=tp[:])])
                    return at

                def gemm_half(at, half, mb):
                    wcol = p_w.tile([128, nkb, 128], BF, tag="wcol")
                    nc.sync.dma_start(out=wcol[:], in_=wb_d[0][:, mb, :, :])
                    for j in range(2):
                        ns = half * 2 + j
                        gps = p_gemm.tile([128, 512], F32, tag="g")
                        for kb in range(nkb):
                            nc.tensor.matmul(
                                out=gps[:],
                                lhsT=wcol[:, kb, :],
                                rhs=at[:, kb, j * 512:(j + 1) * 512],
                                start=(kb == 0), stop=(kb == nkb - 1),
                            )
                        ht = p_ht.tile([128, 512], BF, tag="ht")
                        nc.scalar.activation(out=ht[:], in_=gps[:], func=RELU,
                                             bias=bcol_sb[0][:, mb:mb + 1])
                        nc.sync.dma_start(
                            out=h4T.ap()[mb * 128:(mb + 1) * 128,
                                         ns * 512:(ns + 1) * 512],
                            in_=ht[:],
                        )

                # half 0 aggregation, then half 1 aggregation interleaved
                # with half-0 GEMM (4 weight columns per agg tile), then
                # the remaining half-1 GEMM.  W3 is streamed twice.
                at0 = agg_half(0)
                at1 = p_rhs.tile([128, nkb, 1024], BF, tag="rhs",
                                 name="aggT3_1")
                for tt in range(8):
                    t = 8 + tt
                    agg_group(src_ap, fa, t, 1,
                              [lambda fb, tp, at=at1, tt=tt:
                               nc.vector.tensor_copy(
                                   out=at[:, fb, tt * 128:(tt + 1) * 128],
                                   in_=tp[:])])
                    for mb in range(tt * 4, tt * 4 + 4):
                        gemm_half(at0, 0, mb)
                for mb in range(fo // 128):
                    gemm_half(at1, 1, mb)

            def layer_L4_gemm():
                fa, fo = DIMS[3], DIMS[4]
                nkb = fa // 128
                for ns in range(4):
                    rhs = p_rhs.tile([128, nkb, 512], BF, tag="rhs",
                                     name=f"h4s_{ns}")
                    nc.sync.dma_start(
                        out=rhs[:],
                        in_=h4T.ap().rearrange("(kb p) n -> p kb n", p=128)
                            [:, :, ns * 512:(ns + 1) * 512],
                    )
                    for mb in range(fo // 128):
                        wcol = p_w.tile([128, nkb, 128], BF, tag="wcol")
                        nc.sync.dma_start(out=wcol[:], in_=wb_d[1][:, mb, :, :])
                        gps = p_gemm.tile([128, 512], F32, tag="g")
                        for kb in range(nkb):
                            nc.tensor.matmul(
                                out=gps[:],
                                lhsT=wcol[:, kb, :],
                                rhs=rhs[:, kb, :],
                                start=(kb == 0), stop=(kb == nkb - 1),
                            )
                        ht = p_ht.tile([128, 512], BF, tag="ht")
                        nc.vector.tensor_copy(out=ht[:], in_=gps[:])
                        nst = p_nmst.tile([128, 4, 128], BF, tag="nmst")
                        for bb in range(4):
                            tp = p_tp.tile([128, 128], BF, tag="tp")
                            nc.tensor.transpose(
                                out=tp[:], in_=ht[:, bb * 128:(bb + 1) * 128],
                                identity=ident[:],
                            )
                            nc.vector.tensor_copy(out=nst[:, bb, :], in_=tp[:])
                        nc.sync.dma_start(
                            out=m4_nm.ap()[ns * 512:(ns + 1) * 512,
                                           mb * 128:(mb + 1) * 128]
                                .rearrange("(b p) m -> p b m", p=128),
                            in_=nst[:],
                        )
                    allgather_rows(m4_nm, m4_full, ns, 4)

            def layer_L4_tail():
                fa = DIMS[4]
                nkb = fa // 128

                def agg_to(rhs5, t, tt):
                    agg_group(m4_full.ap(), fa, t, 1,
                              [lambda fb, tp:
                               nc.vector.tensor_scalar(
                                   out=rhs5[:, fb, tt * 128:(tt + 1) * 128],
                                   in0=tp[:],
                                   scalar1=bcol_sb[1][:, fb:fb + 1],
                                   scalar2=0.0,
                                   op0=mybir.AluOpType.add,
                                   op1=mybir.AluOpType.max,
                               )])

                def head(rhs5, nsh, mb):
                    wcol = p_w.tile([128, nkb, 128], BF, tag="wcol")
                    nc.sync.dma_start(out=wcol[:], in_=wb_d[2][:, mb, :, :])
                    gps = p_gemm.tile([128, 1024], F32, tag="g")
                    for kb in range(nkb):
                        for j in range(2):
                            nc.tensor.matmul(
                                out=gps[:, j * 512:(j + 1) * 512],
                                lhsT=wcol[:, kb, :],
                                rhs=rhs5[:, kb, j * 512:(j + 1) * 512],
                                start=(kb == 0), stop=(kb == nkb - 1),
                            )
                    for j in range(2):
                        of = p_of.tile([128, 512], F32, tag="of")
                        nc.vector.tensor_scalar_add(
                            out=of[:], in0=gps[:, j * 512:(j + 1) * 512],
                            scalar1=bcol_sb[2][:, mb:mb + 1])
                        nc.sync.dma_start(
                            out=outT.ap()[mb * 128:(mb + 1) * 128,
                                          nsh * 1024 + j * 512:
                                          nsh * 1024 + (j + 1) * 512],
                            in_=of[:],
                        )

                # half-0 aggregation, then half-1 aggregation interleaved
                # with the half-0 head GEMM (one Wout column per agg tile),
                # then the half-1 head GEMM.  Wout is streamed twice.
                rhs5_0 = p_rhs.tile([128, nkb, 1024], BF, tag="rhs",
                                    name="rhs5_0")
                for tt in range(8):
                    agg_to(rhs5_0, tt, tt)
                rhs5_1 = p_rhs.tile([128, nkb, 1024], BF, tag="rhs",
                                    name="rhs5_1")
                for tt in range(8):
                    agg_to(rhs5_1, 8 + tt, tt)
                    head(rhs5_0, 0, tt)
                for mb in range(CPAD // 128):
                    head(rhs5_1, 1, mb)

            # ---- the network ----
            layer_A(x_bf.ap(), DIMS[0], DIMS[1], 0, h2_nm, h2_full)
            layer_A(h2_full.ap(), DIMS[1], DIMS[2], 1, h3_nm, h3_full)
            layer_L3(h3_full.ap())
            layer_L4_gemm()
            layer_L4_tail()

    nc.compile()
    return nc


# ----------------------------------------------------------------------------
# Host-side preprocessing
# ----------------------------------------------------------------------------

def _balance_perm(dst):
    """Permutation of node ids equalizing edge counts per 128-node tile.

    Greedy first-fit-decreasing over the 128 global dst tiles; with the
    average at E/128 = 512 edges/tile this typically lands every tile at
    exactly 512 (vs ~585 max unbalanced), dropping the scatter chunk
    count by one.  Returns new2old (new node id -> old node id).
    """
    import heapq
    deg = np.bincount(dst, minlength=N)
    order = np.argsort(-deg, kind="stable")
    nbins = N // 128
    bin_cnt = np.zeros(nbins, np.int64)
    bin_sum = np.zeros(nbins, np.int64)
    members = [[] for _ in range(nbins)]
    heap = [(0, b) for b in range(nbins)]
    heapq.heapify(heap)
    for nd in order:
        while True:
            s, b = heapq.heappop(heap)
            if bin_cnt[b] < 128:
                break
        members[b].append(nd)
        bin_cnt[b] += 1
        bin_sum[b] += deg[nd]
        if bin_cnt[b] < 128:
            heapq.heappush(heap, (bin_sum[b], b))
    new2old = np.concatenate([np.asarray(m, np.int64) for m in members])
    return new2old


def _prep_graph(edge_src, edge_dst, edge_weight):
    src = np.asarray(edge_src).astype(np.int64)
    dst = np.asarray(edge_dst).astype(np.int64)
    ew = np.asarray(edge_weight).astype(np.float64)

    # relabel nodes to balance per-tile edge counts
    new2old = _balance_perm(dst)
    perm = np.empty(N, np.int64)
    perm[new2old] = np.arange(N)
    src = perm[src]
    dst = perm[dst]

    deg = np.bincount(dst, weights=ew, minlength=N) + 1.0
    dinv = 1.0 / np.sqrt(deg)
    norm = (dinv[src] * ew * dinv[dst]).astype(np.float32)
    selfc = (dinv * dinv).astype(np.float32)

    gtile = dst // 128                      # global dst tile 0..127
    counts = np.bincount(gtile, minlength=128)
    chunks = int(np.ceil((counts.max() + 128) / 128.0))
    cap = chunks * 128

    order = np.argsort(gtile, kind="stable")
    starts = np.zeros(128, np.int64)
    starts[1:] = np.cumsum(counts)[:-1]
    # position of each (sorted) edge within its tile
    pos_sorted = np.arange(E) - starts[gtile[order]]

    idx_all = np.zeros((128, cap), np.int16)          # [global tile, slot]
    val_all = np.zeros((128, cap), np.float32)
    m_all = np.zeros((128, cap), np.int64)            # dst within tile

    idx_all[gtile[order], pos_sorted] = src[order].astype(np.int16)
    val_all[gtile[order], pos_sorted] = norm[order]
    m_all[gtile[order], pos_sorted] = (dst[order] - gtile[order] * 128)

    # self loops appended after each tile's edges
    nodes = np.arange(N)
    sl_tile = nodes // 128
    sl_pos = counts[sl_tile] + (nodes % 128)
    idx_all[sl_tile, sl_pos] = nodes.astype(np.int16)
    val_all[sl_tile, sl_pos] = selfc
    m_all[sl_tile, sl_pos] = nodes % 128

    # dense scatter matrices S[tile, chunk, k, m]
    s_dense = np.zeros((128, chunks, 128, 128), np.float32)
    tt = np.repeat(np.arange(128), cap)
    pp = np.tile(np.arange(cap), 128)
    s_dense[tt, pp // 128, pp % 128, m_all.reshape(-1)] = val_all.reshape(-1)

    # AG'd tensors land as row blocks: split-s AG block k holds rank r's
    # shard rows [k*NL/s,(k+1)*NL/s) at full rows k*N/s + r*NL/s + ...
    def remap(nodes, nsplit):
        r = nodes // NL
        l = nodes % NL
        blk = l // (NL // nsplit)
        return (blk * (N // nsplit) + r * (NL // nsplit)
                + l % (NL // nsplit)).astype(np.int16)

    idx_quar = remap(idx_all.astype(np.int64), AG_SPLIT)

    # per-core device layouts
    idx_dev = np.zeros((NCORES, 128, NT, chunks * 8), np.int16)
    s_dev = np.zeros((NCORES, 128, NT, chunks, 128), NPBF)
    for r in range(NCORES):
        for t in range(NT):
            g = r * NT + t
            packed = idx_quar[g].reshape(-1, 16).T         # [16, chunks*8]
            idx_dev[r, :, t, :] = np.tile(packed, (8, 1))
            s_dev[r, :, t, :, :] = s_dense[g].transpose(1, 0, 2).astype(NPBF)
    return chunks, idx_dev, s_dev, new2old


def _prep_weights(inputs):
    """Device weight layouts.

    Returns (wa, brow, wb, bcol): form-A panels + bias rows for L1/L2,
    form-B columns + bias cols for L3/L4/head.
    """
    wa, brow = [], []
    for i in (1, 2):
        w = np.asarray(inputs[f"W{i}"], np.float32)
        b = np.asarray(inputs[f"b{i}"], np.float32)
        fa, fo = w.shape
        wd = w.reshape(fa // 128, 128, fo).transpose(1, 0, 2)
        wa.append(np.ascontiguousarray(wd).astype(NPBF))
        brow.append(np.ascontiguousarray(b.reshape(1, fo)).astype(NPBF))

    wout = np.zeros((DIMS[4], CPAD), np.float32)
    wout[:, :C] = np.asarray(inputs["Wout"], np.float32)
    bout = np.zeros(CPAD, np.float32)
    bout[:C] = np.asarray(inputs["bout"], np.float32)

    wb, bcol = [], []
    for w, b in ((np.asarray(inputs["W3"], np.float32),
                  np.asarray(inputs["b3"], np.float32)),
                 (np.asarray(inputs["W4"], np.float32),
                  np.asarray(inputs["b4"], np.float32)),
                 (wout, bout)):
        fa, fo = w.shape
        wd = w.reshape(fa // 128, 128, fo // 128, 128).transpose(1, 2, 0, 3)
        wb.append(np.ascontiguousarray(wd).astype(NPBF))
        bcol.append(np.ascontiguousarray(b.reshape(-1, 128).T).astype(np.float32))
    return wa, brow, wb, bcol


def _run(inputs, trace=False, **kw):
    x = np.asarray(inputs["x"], np.float32)
    chunks, idx_dev, s_dev, new2old = _prep_graph(
        inputs["edge_src"], inputs["edge_dst"], inputs["edge_weight"])
    wa, brow, wb, bcol = _prep_weights(inputs)

    if chunks not in _CACHE:
        _CACHE[chunks] = _build(chunks)
    nc = _CACHE[chunks]

    # x rows: balance-relabel, then permute into the half-split AG layout
    # so layer 1 shares the same (remapped) gather indices as layers 2-4
    s = AG_SPLIT
    nodes = np.arange(N)
    newrow = ((nodes % NL) // (NL // s)) * (N // s) \
        + (nodes // NL) * (NL // s) + (nodes % NL) % (NL // s)
    x_rm = np.empty_like(x)
    x_rm[newrow] = x[new2old]
    x_bf = np.ascontiguousarray(x_rm).astype(NPBF)
    in_maps = []
    for r in range(NCORES):
        m = {"x_bf": x_bf, "idx": idx_dev[r], "smat": s_dev[r]}
        for l in range(2):
            m[f"wa{l}"] = wa[l]
            m[f"brow{l}"] = brow[l]
        for i in range(3):
            m[f"wb{i}"] = wb[i]
            m[f"bcol{i}"] = bcol[i]
        in_maps.append(m)

    res = run_bass_kernel_spmd(nc, in_maps, core_ids=list(range(NCORES)),
                               trace=trace, **kw)
    out_new = np.concatenate(
        [res.results[r]["outT"].T[:, :C] for r in range(NCORES)], axis=0)
    out = np.empty_like(out_new)
    out[new2old] = out_new
    return np.ascontiguousarray(out.astype(np.float32)), res


def kernel(**inputs) -> np.ndarray:
    out, _ = _run(inputs, trace=False)
    return out


# revision 43
# speedup vs baseline: 1.0296x; 1.0296x over previous
"""GCN forward (4x GCNConv + linear head) on 8 Trainium2 NeuronCores.

Sharding: nodes are partitioned across the 8 cores (2048 rows each).
Per layer: AllGather of node-major activations (bf16), per-dst-tile
dma_gather of neighbor rows, scatter-matmul S.T @ gathered (PSUM f32
accumulate), PE-transpose to feature-major, dense GEMM.

Uses the algebraic identity A_hat (h W) = (A_hat h) W to aggregate in the
cheaper feature dimension per layer (layers 1-3 aggregate first, layer 4
aggregates after its GEMM).

Layer GEMM forms:
  - L1/L2 ("form A"): lhsT = transposed-aggregation block [feat,node],
    rhs = full weight panel [feat, fo] resident in SBUF.  Output lands
    node-major directly (no output transposes), bias is pre-loaded into
    PSUM via a K=1 ones-row matmul, relu fused in the ScalarE copy-out.
    Weights are streamed exactly once per layer.
  - L3 ("form B"): lhsT = weight columns, rhs = aggregation (both
    1024-node halves resident), output feature-major h4T for layer 4.
  - L4 ("form B"): lhsT = W4 columns, rhs = h4T node-slices (pair
    resident), node-major output via PE transposes, AllGather, then the
    layer-4 aggregation writes bias+relu'd strips straight into SBUF
    tiles that the output head consumes as its rhs (no DRAM roundtrip).
"""

import os

import numpy as np
import ml_dtypes

try:  # persistent compilation cache: skip walrus recompile across processes
    import jax
    jax.config.update("jax_compilation_cache_dir",
                      os.path.expanduser("~/.cache/jax_gcn_kernel"))
    jax.config.update("jax_persistent_cache_min_compile_time_secs", 10)
except Exception:
    pass

import concourse.bass as bass
import concourse.mybir as mybir
import concourse.tile as tile
from concourse import bacc
from concourse.bass_utils import run_bass_kernel_spmd
from concourse.masks import make_identity

N = 16384
E = 65536
NCORES = 8
NL = N // NCORES            # 2048 nodes per core
NT = NL // 128              # 16 dst tiles per core
DIMS = [512, 1024, 2048, 4096, 2048]
C = 1000
CPAD = 1024                 # output classes padded to a multiple of 128
AG_SPLIT = 4                # AllGather emitted per quarter-shard

BF = mybir.dt.bfloat16
F32 = mybir.dt.float32
I16 = mybir.dt.int16
RELU = mybir.ActivationFunctionType.Relu
COPY = mybir.ActivationFunctionType.Copy
NPBF = ml_dtypes.bfloat16

_CACHE = {}


# ----------------------------------------------------------------------------
# Device program
# ----------------------------------------------------------------------------

def _build(chunks):
    nc = bacc.Bacc("TRN2", target_bir_lowering=False, num_devices=NCORES)

    # ---- kernel I/O ----
    x_bf = nc.dram_tensor("x_bf", [N, DIMS[0]], BF, kind="ExternalInput")
    idx_d = nc.dram_tensor("idx", [128, NT, chunks * 8], I16, kind="ExternalInput")
    s_d = nc.dram_tensor("smat", [128, NT, chunks, 128], BF, kind="ExternalInput")
    # form-A weight panels [feat-part, fa/128, fo] for layers 1-2
    wa_d = [
        nc.dram_tensor("wa0", [128, DIMS[0] // 128, DIMS[1]], BF,
                       kind="ExternalInput"),
        nc.dram_tensor("wa1", [128, DIMS[1] // 128, DIMS[2]], BF,
                       kind="ExternalInput"),
    ]
    # form-A panel for W4 (layer-4 GEMM streams it as the moving operand)
    wa2_d = nc.dram_tensor("wa2", [128, DIMS[3] // 128, DIMS[4]], BF,
                           kind="ExternalInput")
    # bias rows [1, fo] for layers 1-2 (preloaded into PSUM via K=1 matmul)
    brow_d = [
        nc.dram_tensor("brow0", [1, DIMS[1]], BF, kind="ExternalInput"),
        nc.dram_tensor("brow1", [1, DIMS[2]], BF, kind="ExternalInput"),
    ]
    # form-B weight columns [part, fo/128, fa/128, 128] for L3, L4, head
    wb_dims = [(DIMS[2], DIMS[3]), (DIMS[3], DIMS[4]), (DIMS[4], CPAD)]
    wb_d = []
    bcol_d = []
    for i, (fa, fo) in enumerate(wb_dims):
        wb_d.append(nc.dram_tensor(f"wb{i}", [128, fo // 128, fa // 128, 128],
                                   BF, kind="ExternalInput"))
        bcol_d.append(nc.dram_tensor(f"bcol{i}", [128, fo // 128], F32,
                                     kind="ExternalInput"))
    outT = nc.dram_tensor("outT", [CPAD, NL], F32, kind="ExternalOutput")

    # ---- internal DRAM ----
    h2_nm = nc.dram_tensor("h2_nm", [NL, DIMS[1]], BF)
    h2_full = nc.dram_tensor("h2_full", [N, DIMS[1]], BF, addr_space="Shared")
    h3_nm = nc.dram_tensor("h3_nm", [NL, DIMS[2]], BF)
    h3_full = nc.dram_tensor("h3_full", [N, DIMS[2]], BF, addr_space="Shared")
    h4T = nc.dram_tensor("h4T", [DIMS[3], NL], BF)
    m4_nm = nc.dram_tensor("m4_nm", [NL, DIMS[4]], BF)
    m4_full = nc.dram_tensor("m4_full", [N, DIMS[4]], BF, addr_space="Shared")

    rg = [list(range(NCORES))]

    with tile.TileContext(nc) as tc:
        with (
            tc.tile_pool(name="const", bufs=1) as p_const,
            tc.tile_pool(name="gath", bufs=2) as p_gath,
            tc.tile_pool(name="rhs", bufs=2) as p_rhs,
            tc.tile_pool(name="wrhs", bufs=1) as p_wrhs,
            tc.tile_pool(name="aggw", bufs=4 if chunks <= 5 else 3) as p_aggw,
            tc.tile_pool(name="wcol", bufs=2 if chunks <= 5 else 1) as p_w,
            tc.tile_pool(name="nm", bufs=4 if chunks <= 5 else 2) as p_nm,
            tc.tile_pool(name="nmst", bufs=2 if chunks <= 5 else 1) as p_nmst,
            tc.tile_pool(name="ht", bufs=6 if chunks <= 5 else 2) as p_ht,
            tc.tile_pool(name="of", bufs=1) as p_of,
            tc.tile_pool(name="tp", bufs=2, space="PSUM") as p_tp,
            tc.tile_pool(name="aggps", bufs=2, space="PSUM") as p_agg,
            tc.tile_pool(name="gemmps", bufs=2, space="PSUM") as p_gemm,
        ):
            # ---- constants ----
            idx_sb = p_const.tile([128, NT, chunks * 8], I16)
            nc.sync.dma_start(out=idx_sb[:], in_=idx_d[:])
            s_sb = p_const.tile([128, NT, chunks, 128], BF)
            nc.sync.dma_start(out=s_sb[:], in_=s_d[:])
            ident = p_const.tile([128, 128], BF)
            make_identity(nc, ident[:])
            ones1 = p_const.tile([1, 128], BF)
            nc.vector.memset(ones1[:], 1.0)
            brow_sb = []
            for l in range(2):
                bt = p_const.tile([1, DIMS[l + 1]], BF, tag=f"brow{l}")
                nc.sync.dma_start(out=bt[:], in_=brow_d[l][:])
                brow_sb.append(bt)
            bcol_sb = []
            for i, (fa, fo) in enumerate(wb_dims):
                bt = p_const.tile([128, fo // 128], F32, tag=f"bcol{i}")
                nc.sync.dma_start(out=bt[:], in_=bcol_d[i][:])
                bcol_sb.append(bt)

            def allgather_rows(nm, full, k, nsplit):
                # AG of shard rows [k*NL/s,(k+1)*NL/s) -> full rows
                # [k*N/s,(k+1)*N/s). Gather indices are host-remapped to this
                # block-of-row-blocks layout.
                rl = NL // nsplit
                gl = N // nsplit
                nc.gpsimd.collective_compute(
                    "AllGather", mybir.AluOpType.bypass, replica_groups=rg,
                    ins=[nm.ap()[k * rl:(k + 1) * rl, :].opt()],
                    outs=[full.ap()[k * gl:(k + 1) * gl, :].opt()],
                )

            def agg_group(src_ap, fa, t0, nt, sinks):
                """Aggregate dst tiles t0..t0+nt-1 with a single dma_gather.

                sinks[i](fb, tp) receives the transposed 128x128 PSUM block
                for features [fb*128,(fb+1)*128) of tile t0+i.
                """
                gath = p_gath.tile([128, nt * chunks, fa], BF, tag="gath")
                nc.gpsimd.dma_gather(
                    gath[:], src_ap, idx_sb[:, t0:t0 + nt, :],
                    nt * chunks * 128, nt * chunks * 128, fa,
                )
                for i in range(nt):
                    t = t0 + i
                    for q in range(fa // 512):
                        aps = p_agg.tile([128, 512], F32, tag="aggps")
                        for c in range(chunks):
                            nc.tensor.matmul(
                                out=aps[:],
                                lhsT=s_sb[:, t, c, :],
                                rhs=gath[:, i * chunks + c,
                                         q * 512:(q + 1) * 512],
                                start=(c == 0), stop=(c == chunks - 1),
                            )
                        nm = p_nm.tile([128, 512], BF, tag="aggnm")
                        nc.vector.tensor_copy(out=nm[:], in_=aps[:])
                        for bb in range(4):
                            fb = q * 4 + bb
                            tp = p_tp.tile([128, 128], BF, tag="tp")
                            nc.tensor.transpose(
                                out=tp[:], in_=nm[:, bb * 128:(bb + 1) * 128],
                                identity=ident[:],
                            )
                            sinks[i](fb, tp)

            def layer_A(src_ap, fa, fo, li, out_nm, out_full):
                """Form-A layer (L1/L2): rolling agg -> GEMM per node tile.

                Gathers run two dst tiles per dma_gather.  The half-shard
                AllGather triggers are emitted only after the last gather of
                the layer so a blocked trigger never stalls this layer's
                gather queue.
                """
                w_rhs = p_wrhs.tile([128, fa // 128, fo], BF, tag="wrhs",
                                    name=f"wrhs{li}")
                nc.sync.dma_start(out=w_rhs[:], in_=wa_d[li][:])
                nkb = fa // 128
                nfh = fo // 1024

                def gemm_tile(t):
                    for fh in range(nfh):
                        gps = p_gemm.tile([128, 1024], F32, tag="g")
                        for j in range(2):
                            nc.tensor.matmul(
                                out=gps[:, j * 512:(j + 1) * 512],
                                lhsT=ones1[:],
                                rhs=brow_sb[li][:, fh * 1024 + j * 512:
                                                fh * 1024 + (j + 1) * 512],
                                start=True, stop=False,
                            )
                        for kb in range(nkb):
                            for j in range(2):
                                nc.tensor.matmul(
                                    out=gps[:, j * 512:(j + 1) * 512],
                                    lhsT=aggT[t][:, kb, :],
                                    rhs=w_rhs[:, kb, fh * 1024 + j * 512:
                                              fh * 1024 + (j + 1) * 512],
                                    start=False, stop=(kb == nkb - 1),
                                )
                        for j in range(2):
                            ht = p_ht.tile([128, 512], BF, tag="ht")
                            nc.scalar.activation(
                                out=ht[:], in_=gps[:, j * 512:(j + 1) * 512],
                                func=RELU)
                            nc.sync.dma_start(
                                out=out_nm.ap()[t * 128:(t + 1) * 128,
                                                fh * 1024 + j * 512:
                                                fh * 1024 + (j + 1) * 512],
                                in_=ht[:],
                            )

                aggT = {}

                def make_sink(t):
                    at = p_aggw.tile([128, nkb, 128], BF, tag="aggw")
                    aggT[t] = at
                    return lambda fb, tp: nc.vector.tensor_copy(
                        out=at[:, fb, :], in_=tp[:])

                # Quarter-k AG triggers are emitted on the gpsimd queue only
                # once the producing GEMM tile (4k+3) is safely complete, so
                # a waiting trigger never stalls the remaining gathers.
                for t in range(NT):
                    agg_group(src_ap, fa, t, 1, [make_sink(t)])
                    if t >= 2:
                        gemm_tile(t - 2)
                    if t == 11:
                        allgather_rows(out_nm, out_full, 0, 4)
                    elif t == 13:
                        allgather_rows(out_nm, out_full, 1, 4)
                    elif t == 15:
                        allgather_rows(out_nm, out_full, 2, 4)
                gemm_tile(NT - 2)
                gemm_tile(NT - 1)
                allgather_rows(out_nm, out_full, 3, 4)

            def layer_L3(src_ap):
                fa, fo = DIMS[2], DIMS[3]
                nkb = fa // 128

                def agg_half(half):
                    at = p_rhs.tile([128, nkb, 1024], BF, tag="rhs",
                                    name=f"aggT3_{half}")
                    for tt in range(8):
                        t = half * 8 + tt
                        agg_group(src_ap, fa, t, 1,
                                  [lambda fb, tp, at=at, tt=tt:
                                   nc.vector.tensor_copy(
                                       out=at[:, fb, tt * 128:(tt + 1) * 128],
                                       in_=tp[:])])
                    return at

                def gemm_half(at, half, mb):
                    wcol = p_w.tile([128, nkb, 128], BF, tag="wcol")
                    nc.sync.dma_start(out=wcol[:], in_=wb_d[0][:, mb, :, :])
                    for j in range(2):
                        ns = half * 2 + j
                        gps = p_gemm.tile([128, 512], F32, tag="g")
                        for kb in range(nkb):
                            nc.tensor.matmul(
                                out=gps[:],
                                lhsT=wcol[:, kb, :],
                                rhs=at[:, kb, j * 512:(j + 1) * 512],
                                start=(kb == 0), stop=(kb == nkb - 1),
                            )
                        ht = p_ht.tile([128, 512], BF, tag="ht")
                        nc.scalar.activation(out=ht[:], in_=gps[:], func=RELU,
                                             bias=bcol_sb[0][:, mb:mb + 1])
                        nc.sync.dma_start(
                            out=h4T.ap()[mb * 128:(mb + 1) * 128,
                                         ns * 512:(ns + 1) * 512],
                            in_=ht[:],
                        )

                # half 0 aggregation, then half 1 aggregation interleaved
                # with half-0 GEMM (4 weight columns per agg tile), then
                # the remaining half-1 GEMM.  W3 is streamed twice.
                at0 = agg_half(0)
                at1 = p_rhs.tile([128, nkb, 1024], BF, tag="rhs",
                                 name="aggT3_1")
                for tt in range(8):
                    t = 8 + tt
                    agg_group(src_ap, fa, t, 1,
                              [lambda fb, tp, at=at1, tt=tt:
                               nc.vector.tensor_copy(
                                   out=at[:, fb, tt * 128:(tt + 1) * 128],
                                   in_=tp[:])])
                    for mb in range(tt * 4, tt * 4 + 4):
                        gemm_half(at0, 0, mb)
                for mb in range(fo // 128):
                    gemm_half(at1, 1, mb)

            def layer_L4_gemm():
                # Form A: lhsT = h4T 128-col blocks sliced from the resident
                # node-slice tile, rhs = W4 panels [feat, 512] streamed and
                # double-buffered by alternating the two 32KB pools.  Output
                # lands node-major directly (no transposes).
                fa, fo = DIMS[3], DIMS[4]
                nkb = fa // 128
                for g in range(4):
                    h4s = p_rhs.tile([128, nkb, 512], BF, tag="rhs",
                                     name=f"h4s_{g}")
                    nc.sync.dma_start(
                        out=h4s[:],
                        in_=h4T.ap().rearrange("(kb p) n -> p kb n", p=128)
                            [:, :, g * 512:(g + 1) * 512],
                    )
                    for fo_blk in range(4):
                        wp_pool = p_rhs if fo_blk % 2 == 0 else p_wrhs
                        wp_tag = "rhs" if fo_blk % 2 == 0 else "wrhs"
                        wpan = wp_pool.tile([128, nkb, 512], BF, tag=wp_tag,
                                            name=f"w4p_{g}_{fo_blk}")
                        nc.sync.dma_start(
                            out=wpan[:],
                            in_=wa2_d.ap()[:, :,
                                           fo_blk * 512:(fo_blk + 1) * 512],
                        )
                        for i in range(4):
                            nt = g * 4 + i
                            gps = p_gemm.tile([128, 512], F32, tag="g")
                            for kb in range(nkb):
                                nc.tensor.matmul(
                                    out=gps[:],
                                    lhsT=h4s[:, kb, i * 128:(i + 1) * 128],
                                    rhs=wpan[:, kb, :],
                                    start=(kb == 0), stop=(kb == nkb - 1),
                                )
                            ht = p_ht.tile([128, 512], BF, tag="ht")
                            nc.vector.tensor_copy(out=ht[:], in_=gps[:])
                            nc.sync.dma_start(
                                out=m4_nm.ap()[nt * 128:(nt + 1) * 128,
                                               fo_blk * 512:(fo_blk + 1) * 512],
                                in_=ht[:],
                            )
                    allgather_rows(m4_nm, m4_full, g, 4)

            def layer_L4_tail():
                fa = DIMS[4]
                nkb = fa // 128

                def agg_to(rhs5, t, tt):
                    agg_group(m4_full.ap(), fa, t, 1,
                              [lambda fb, tp:
                               nc.vector.tensor_scalar(
                                   out=rhs5[:, fb, tt * 128:(tt + 1) * 128],
                                   in0=tp[:],
                                   scalar1=bcol_sb[1][:, fb:fb + 1],
                                   scalar2=0.0,
                                   op0=mybir.AluOpType.add,
                                   op1=mybir.AluOpType.max,
                               )])

                def head(rhs5, nsh, mb):
                    wcol = p_w.tile([128, nkb, 128], BF, tag="wcol")
                    nc.sync.dma_start(out=wcol[:], in_=wb_d[2][:, mb, :, :])
                    gps = p_gemm.tile([128, 1024], F32, tag="g")
                    for kb in range(nkb):
                        for j in range(2):
                            nc.tensor.matmul(
                                out=gps[:, j * 512:(j + 1) * 512],
                                lhsT=wcol[:, kb, :],
                                rhs=rhs5[:, kb, j * 512:(j + 1) * 512],
                                start=(kb == 0), stop=(kb == nkb - 1),
                            )
                    for j in range(2):
                        of = p_of.tile([128, 512], F32, tag="of")
                        nc.vector.tensor_scalar_add(
                            out=of[:], in0=gps[:, j * 512:(j + 1) * 512],
                            scalar1=bcol_sb[2][:, mb:mb + 1])
                        nc.sync.dma_start(
                            out=outT.ap()[mb * 128:(mb + 1) * 128,
                                          nsh * 1024 + j * 512:
                                          nsh * 1024 + (j + 1) * 512],
                            in_=of[:],
                        )

                # half-0 aggregation, then half-1 aggregation interleaved
                # with the half-0 head GEMM (one Wout column per agg tile),
                # then the half-1 head GEMM.  Wout is streamed twice.
                rhs5_0 = p_rhs.tile([128, nkb, 1024], BF, tag="rhs",
                                    name="rhs5_0")
                for tt in range(8):
                    agg_to(rhs5_0, tt, tt)
                rhs5_1 = p_rhs.tile([128, nkb, 1024], BF, tag="rhs",
                                    name="rhs5_1")
                for tt in range(8):
                    agg_to(rhs5_1, 8 + tt, tt)
                    head(rhs5_0, 0, tt)
                for mb in range(CPAD // 128):
                    head(rhs5_1, 1, mb)

            # ---- the network ----
            layer_A(x_bf.ap(), DIMS[0], DIMS[1], 0, h2_nm, h2_full)
            layer_A(h2_full.ap(), DIMS[1], DIMS[2], 1, h3_nm, h3_full)
            layer_L3(h3_full.ap())
            layer_L4_gemm()
            layer_L4_tail()

    nc.compile()
    return nc


# ----------------------------------------------------------------------------
# Host-side preprocessing
# ----------------------------------------------------------------------------

def _balance_perm(dst):
    """Permutation of node ids equalizing edge counts per 128-node tile.

    Greedy first-fit-decreasing over the 128 global dst tiles; with the
    average at E/128 = 512 edges/tile this typically lands every tile at
    exactly 512 (vs ~585 max unbalanced), dropping the scatter chunk
    count by one.  Returns new2old (new node id -> old node id).
    """
    import heapq
    deg = np.bincount(dst, minlength=N)
    order = np.argsort(-deg, kind="stable")
    nbins = N // 128
    bin_cnt = np.zeros(nbins, np.int64)
    bin_sum = np.zeros(nbins, np.int64)
    members = [[] for _ in range(nbins)]
    heap = [(0, b) for b in range(nbins)]
    heapq.heapify(heap)
    for nd in order:
        while True:
            s, b = heapq.heappop(heap)
            if bin_cnt[b] < 128:
                break
        members[b].append(nd)
        bin_cnt[b] += 1
        bin_sum[b] += deg[nd]
        if bin_cnt[b] < 128:
            heapq.heappush(heap, (bin_sum[b], b))
    new2old = np.concatenate([np.asarray(m, np.int64) for m in members])
    return new2old


def _prep_graph(edge_src, edge_dst, edge_weight):
    src = np.asarray(edge_src).astype(np.int64)
    dst = np.asarray(edge_dst).astype(np.int64)
    ew = np.asarray(edge_weight).astype(np.float64)

    # relabel nodes to balance per-tile edge counts
    new2old = _balance_perm(dst)
    perm = np.empty(N, np.int64)
    perm[new2old] = np.arange(N)
    src = perm[src]
    dst = perm[dst]

    deg = np.bincount(dst, weights=ew, minlength=N) + 1.0
    dinv = 1.0 / np.sqrt(deg)
    norm = (dinv[src] * ew * dinv[dst]).astype(np.float32)
    selfc = (dinv * dinv).astype(np.float32)

    gtile = dst // 128                      # global dst tile 0..127
    counts = np.bincount(gtile, minlength=128)
    chunks = int(np.ceil((counts.max() + 128) / 128.0))
    cap = chunks * 128

    order = np.argsort(gtile, kind="stable")
    starts = np.zeros(128, np.int64)
    starts[1:] = np.cumsum(counts)[:-1]
    # position of each (sorted) edge within its tile
    pos_sorted = np.arange(E) - starts[gtile[order]]

    idx_all = np.zeros((128, cap), np.int16)          # [global tile, slot]
    val_all = np.zeros((128, cap), np.float32)
    m_all = np.zeros((128, cap), np.int64)            # dst within tile

    idx_all[gtile[order], pos_sorted] = src[order].astype(np.int16)
    val_all[gtile[order], pos_sorted] = norm[order]
    m_all[gtile[order], pos_sorted] = (dst[order] - gtile[order] * 128)

    # self loops appended after each tile's edges
    nodes = np.arange(N)
    sl_tile = nodes // 128
    sl_pos = counts[sl_tile] + (nodes % 128)
    idx_all[sl_tile, sl_pos] = nodes.astype(np.int16)
    val_all[sl_tile, sl_pos] = selfc
    m_all[sl_tile, sl_pos] = nodes % 128

    # dense scatter matrices S[tile, chunk, k, m]
    s_dense = np.zeros((128, chunks, 128, 128), np.float32)
    tt = np.repeat(np.arange(128), cap)
    pp = np.tile(np.arange(cap), 128)
    s_dense[tt, pp // 128, pp % 128, m_all.reshape(-1)] = val_all.reshape(-1)

    # AG'd tensors land as row blocks: split-s AG block k holds rank r's
    # shard rows [k*NL/s,(k+1)*NL/s) at full rows k*N/s + r*NL/s + ...
    def remap(nodes, nsplit):
        r = nodes // NL
        l = nodes % NL
        blk = l // (NL // nsplit)
        return (blk * (N // nsplit) + r * (NL // nsplit)
                + l % (NL // nsplit)).astype(np.int16)

    idx_quar = remap(idx_all.astype(np.int64), AG_SPLIT)

    # per-core device layouts
    idx_dev = np.zeros((NCORES, 128, NT, chunks * 8), np.int16)
    s_dev = np.zeros((NCORES, 128, NT, chunks, 128), NPBF)
    for r in range(NCORES):
        for t in range(NT):
            g = r * NT + t
            packed = idx_quar[g].reshape(-1, 16).T         # [16, chunks*8]
            idx_dev[r, :, t, :] = np.tile(packed, (8, 1))
            s_dev[r, :, t, :, :] = s_dense[g].transpose(1, 0, 2).astype(NPBF)
    return chunks, idx_dev, s_dev, new2old


def _prep_weights(inputs):
    """Device weight layouts.

    Returns (wa, brow, wb, bcol): form-A panels + bias rows for L1/L2,
    form-B columns + bias cols for L3/L4/head.
    """
    wa, brow = [], []
    for i in (1, 2):
        w = np.asarray(inputs[f"W{i}"], np.float32)
        b = np.asarray(inputs[f"b{i}"], np.float32)
        fa, fo = w.shape
        wd = w.reshape(fa // 128, 128, fo).transpose(1, 0, 2)
        wa.append(np.ascontiguousarray(wd).astype(NPBF))
        brow.append(np.ascontiguousarray(b.reshape(1, fo)).astype(NPBF))
    w4 = np.asarray(inputs["W4"], np.float32)
    w4d = w4.reshape(w4.shape[0] // 128, 128, w4.shape[1]).transpose(1, 0, 2)
    wa.append(np.ascontiguousarray(w4d).astype(NPBF))

    wout = np.zeros((DIMS[4], CPAD), np.float32)
    wout[:, :C] = np.asarray(inputs["Wout"], np.float32)
    bout = np.zeros(CPAD, np.float32)
    bout[:C] = np.asarray(inputs["bout"], np.float32)

    wb, bcol = [], []
    for w, b in ((np.asarray(inputs["W3"], np.float32),
                  np.asarray(inputs["b3"], np.float32)),
                 (np.asarray(inputs["W4"], np.float32),
                  np.asarray(inputs["b4"], np.float32)),
                 (wout, bout)):
        fa, fo = w.shape
        wd = w.reshape(fa // 128, 128, fo // 128, 128).transpose(1, 2, 0, 3)
        wb.append(np.ascontiguousarray(wd).astype(NPBF))
        bcol.append(np.ascontiguousarray(b.reshape(-1, 128).T).astype(np.float32))
    return wa, brow, wb, bcol


def _run(inputs, trace=False, **kw):
    x = np.asarray(inputs["x"], np.float32)
    chunks, idx_dev, s_dev, new2old = _prep_graph(
        inputs["edge_src"], inputs["edge_dst"], inputs["edge_weight"])
    wa, brow, wb, bcol = _prep_weights(inputs)

    if chunks not in _CACHE:
        _CACHE[chunks] = _build(chunks)
    nc = _CACHE[chunks]

    # x rows: balance-relabel, then permute into the half-split AG layout
    # so layer 1 shares the same (remapped) gather indices as layers 2-4
    s = AG_SPLIT
    nodes = np.arange(N)
    newrow = ((nodes % NL) // (NL // s)) * (N // s) \
        + (nodes // NL) * (NL // s) + (nodes % NL) % (NL // s)
    x_rm = np.empty_like(x)
    x_rm[newrow] = x[new2old]
    x_bf = np.ascontiguousarray(x_rm).astype(NPBF)
    in_maps = []
    for r in range(NCORES):
        m = {"x_bf": x_bf, "idx": idx_dev[r], "smat": s_dev[r]}
        for l in range(2):
            m[f"wa{l}"] = wa[l]
            m[f"brow{l}"] = brow[l]
        m["wa2"] = wa[2]
        for i in range(3):
            m[f"wb{i}"] = wb[i]
            m[f"bcol{i}"] = bcol[i]
        in_maps.append(m)

    res = run_bass_kernel_spmd(nc, in_maps, core_ids=list(range(NCORES)),
                               trace=trace, **kw)
    out_new = np.concatenate(
        [res.results[r]["outT"].T[:, :C] for r in range(NCORES)], axis=0)
    out = np.empty_like(out_new)
    out[new2old] = out_new
    return np.ascontiguousarray(out.astype(np.float32)), res


def kernel(**inputs) -> np.ndarray:
    out, _ = _run(inputs, trace=False)
    return out
